# revision 14
# baseline (speedup 1.0000x reference)
"""Trainium2 Bass kernel for nn_MoEPredictor (moe_routing).

Data-parallel over batch across 8 NeuronCores: each core gets B/8=1024 batch
rows and the full (replicated) router + expert weights, computes its shard of
trajectories / scores / router_probs on-device, and the host concatenates.

Per-core layout: activations are feature-major [feat, token] in SBUF so the
expert MLP weights stay stationary in the PE array; matmuls run as float32r
(full PE rate at free-dim >= 256). The router runs in plain float32 so the
top-2 expert selection matches the fp32 reference. Top-2 gating (max, masked
second max, 2-way softmax) is computed on-device with DVE/ACT ops, and the
dense combine sum_e w[b,e] * expert_e(x) is applied by scaling each expert's
head outputs with partition-broadcast gate-weight tiles.
"""

import numpy as np
from contextlib import ExitStack

# ---- problem constants (hardcoded per contract) ----
NCORES = 8
B_FULL = 8192
B = B_FULL // NCORES  # 1024 batch rows per core
M = 6                 # modes
D = 128
E = 6                 # experts
H = 256               # trajectory head hidden
S1 = 128              # score head hidden 1
S2 = 64               # score head hidden 2
FUT2 = 120            # FUT*2
NT = B * M            # 6144 tokens per core
CH = 1024             # token chunk
NCH = NT // CH        # 6
BCH = 512             # matmul moving free dim

USE_F32R = True

_CACHE = {}


def _build_nc():
    import concourse.bass as bass
    import concourse.tile as tile
    from concourse import bacc, mybir
    from concourse.masks import make_identity

    f32 = mybir.dt.float32
    f32r = mybir.dt.float32r if USE_F32R else f32
    ts = bass.ts
    Alu = mybir.AluOpType
    Act = mybir.ActivationFunctionType

    def rcast(ap):
        return ap.bitcast(f32r) if USE_F32R else ap

    nc = bacc.Bacc("TRN2", target_bir_lowering=False, debug=False,
                   num_devices=NCORES)

    # ---- DRAM I/O ----
    mf = nc.dram_tensor("mode_features", (B, M, D), f32, kind="ExternalInput").ap()
    dW = {}
    for name, shape in [
        ("Wr1", (D, 256)), ("br1", (256,)), ("Wr2", (256, 128)), ("br2", (128,)),
        ("Wr3", (128, E)), ("br3", (E,)),
        ("Wt1", (E, D, H)), ("bt1", (E, H)), ("Wt2", (E, H, H)), ("bt2", (E, H)),
        ("Wt3", (E, H, FUT2)), ("bt3", (E, FUT2)),
        ("Ws1", (E, D, S1)), ("bs1", (E, S1)), ("Ws2", (E, S1, S2)),
        ("bs2", (E, S2)), ("Ws3", (E, S2, 1)), ("bs3", (E, 1)),
    ]:
        dW[name] = nc.dram_tensor(name, shape, f32, kind="ExternalInput").ap()

    o_traj = nc.dram_tensor("out_traj", (B, M, FUT2), f32, kind="ExternalOutput").ap()
    o_scores = nc.dram_tensor("out_scores", (B, M), f32, kind="ExternalOutput").ap()
    o_probs = nc.dram_tensor("out_probs", (B, E), f32, kind="ExternalOutput").ap()

    mf_flat = mf.rearrange("b m d -> (b m) d")          # [NT, D]
    o_traj_flat = o_traj.rearrange("b m o -> (b m) o")  # [NT, FUT2]
    o_scores_flat = o_scores.rearrange("b m -> (b m)")  # [NT]

    with tile.TileContext(nc) as tc, ExitStack() as ctx:
        cpool = ctx.enter_context(tc.tile_pool(name="const", bufs=1))
        xpool = ctx.enter_context(tc.tile_pool(name="xstage", bufs=4))
        h1pool = ctx.enter_context(tc.tile_pool(name="h1", bufs=2))
        h2pool = ctx.enter_context(tc.tile_pool(name="h2", bufs=2))
        hs1pool = ctx.enter_context(tc.tile_pool(name="hs1", bufs=2))
        hs2pool = ctx.enter_context(tc.tile_pool(name="hs2", bufs=2))
        wbpool = ctx.enter_context(tc.tile_pool(name="wb", bufs=2))
        accpool = ctx.enter_context(tc.tile_pool(name="acc", bufs=2))
        tmppool = ctx.enter_context(tc.tile_pool(name="tmp", bufs=2))
        zpool = ctx.enter_context(tc.tile_pool(name="zpsum", bufs=2, space="PSUM"))
        t3pool = ctx.enter_context(tc.tile_pool(name="t3psum", bufs=1, space="PSUM"))
        s3pool = ctx.enter_context(tc.tile_pool(name="s3psum", bufs=1, space="PSUM"))
        dpool = ctx.enter_context(tc.tile_pool(name="dram", bufs=1, space="DRAM"))

        # ---- constants / weights to SBUF ----
        ident = cpool.tile([128, 128], f32, tag="ident")
        make_identity(nc, ident[:])

        def load(shape, src_ap, tag):
            t = cpool.tile(shape, f32, tag=tag)
            nc.sync.dma_start(t[:], src_ap)
            return t

        wstagepool = ctx.enter_context(tc.tile_pool(name="wstage", bufs=2))

        def loadr(shape, src_aps, tag):
            """Load f32 weights and round into an f32r tile (required: every
            operand of an fp32r matmul must be produced as fp32r)."""
            stage = wstagepool.tile(list(shape), f32, tag="wstage", name="wstage")
            for dst_sl, src_ap in src_aps:
                nc.sync.dma_start(stage[dst_sl] if dst_sl else stage[:], src_ap)
            t = cpool.tile(shape, f32r, tag=tag, name=tag)
            nc.vector.tensor_copy(t[:], stage[:])
            return t

        wr1 = load([128, 256], dW["Wr1"][:], "wr1")
        wr2 = cpool.tile([128, 256], f32, tag="wr2")  # [k, 2*128] k-chunks
        for k in range(2):
            nc.sync.dma_start(wr2[:, ts(k, 128)], dW["Wr2"][ts(k, 128), :])
        wr3 = load([128, E], dW["Wr3"][:], "wr3")
        br1 = cpool.tile([128, 2], f32, tag="br1")
        for m in range(2):
            nc.sync.dma_start(br1[:, m : m + 1], dW["br1"][ts(m, 128)][:, None])
        br2 = load([128, 1], dW["br2"][:][:, None], "br2")
        br3 = load([E, 1], dW["br3"][:][:, None], "br3")

        wt1, wt2, wt3, ws1, ws2, bt1, bt2, bs1 = [], [], [], [], [], [], [], []
        for e in range(E):
            wt1.append(loadr([128, 256], [(None, dW["Wt1"][e])], f"wt1_{e}"))
            wt2.append(loadr([128, 512],
                             [((slice(None), slice(256 * k, 256 * k + 256)),
                               dW["Wt2"][e, ts(k, 128), :]) for k in range(2)],
                             f"wt2_{e}"))
            wt3.append(loadr([128, 240],
                             [((slice(None), slice(120 * k, 120 * k + 120)),
                               dW["Wt3"][e, ts(k, 128), :]) for k in range(2)],
                             f"wt3_{e}"))
            ws1.append(loadr([128, 128], [(None, dW["Ws1"][e])], f"ws1_{e}"))
            ws2.append(loadr([128, 64], [(None, dW["Ws2"][e])], f"ws2_{e}"))
            bt1.append(load([128, 2], dW["bt1"][e].rearrange("(m p) -> p m", p=128),
                            f"bt1_{e}"))
            bt2.append(load([128, 2], dW["bt2"][e].rearrange("(m p) -> p m", p=128),
                            f"bt2_{e}"))
            bs1.append(load([128, 1], dW["bs1"][e][:, None], f"bs1_{e}"))
        ws3, bs2 = [], []
        for e in range(E):
            ws3.append(loadr([64, 1], [(None, dW["Ws3"][e])], f"ws3_{e}"))
            t = cpool.tile([64, 1], f32, tag=f"bs2_{e}", name=f"bs2_{e}")
            nc.sync.dma_start(t[:], dW["bs2"][e][:, None])
            bs2.append(t)
        # [bt3 | bs3] stacked for the fp32 gate-weighted bias matmul
        btst = cpool.tile([E, FUT2 + 1], f32, tag="btst")
        nc.sync.dma_start(btst[:, 0:FUT2], dW["bt3"][:])
        nc.sync.dma_start(btst[:, FUT2 : FUT2 + 1], dW["bs3"][:])

        # ---- X load + transpose to feature-major [D, NT] ----
        x_fm = cpool.tile([128, NT], f32r, tag="x_fm")
        for j in range(NT // 1024):
            zt = zpool.tile([128, 1024], f32, tag="z")
            for k in range(8):
                xs = xpool.tile([128, 128], f32, tag="xs")
                nc.sync.dma_start(xs[:], mf_flat[ts(8 * j + k, 128), :])
                nc.tensor.transpose(zt[:, ts(k, 128)], xs[:], ident[:])
            nc.vector.tensor_copy(x_fm[:, ts(j, 1024)], zt[:])

        # ---- router (plain fp32 matmuls for selection fidelity) ----
        ctx_sum = cpool.tile([128, B], f32, tag="ctx")
        nc.vector.reduce_sum(ctx_sum[:],
                             x_fm[:].bitcast(f32).rearrange("p (b m) -> p b m", m=M),
                             axis=mybir.AxisListType.X)
        hr1 = h1pool.tile([128, 2, B], f32, tag="hr1", bufs=1)
        for m in range(2):
            zt = zpool.tile([128, B], f32, tag="z")
            for s in range(B // BCH):
                nc.tensor.matmul(zt[:, ts(s, BCH)], wr1[:, ts(m, 128)],
                                 ctx_sum[:, ts(s, BCH)], start=True, stop=True)
            nc.scalar.activation(hr1[:, m, :], zt[:], Act.Gelu,
                                 bias=br1[:, m : m + 1], scale=1.0 / M)
        hr2 = h2pool.tile([128, B], f32, tag="hr2", bufs=1)
        zt = zpool.tile([128, B], f32, tag="z")
        for s in range(B // BCH):
            for k in range(2):
                nc.tensor.matmul(zt[:, ts(s, BCH)], wr2[:, ts(k, 128)],
                                 hr1[:, k, ts(s, BCH)], start=(k == 0), stop=(k == 1))
        nc.scalar.activation(hr2[:], zt[:], Act.Gelu, bias=br2[:, 0:1])
        logits = cpool.tile([E, B], f32, tag="logits")
        zt = zpool.tile([E, B], f32, tag="z")
        for s in range(B // BCH):
            nc.tensor.matmul(zt[:, ts(s, BCH)], wr3[:], hr2[:, ts(s, BCH)],
                             start=True, stop=True)
        nc.scalar.activation(logits[:], zt[:], Act.Identity, bias=br3[:, 0:1])

        # ---- gating: transpose logits to [b-part, jtile, e], top-2 softmax ----
        NJ = B // 128  # 8
        zt = zpool.tile([128, NJ * E], f32, tag="z")
        for j in range(NJ):
            nc.tensor.transpose(zt[:, ts(j, E)], logits[:, ts(j, 128)],
                                ident[0:E, 0:E])
        lt = cpool.tile([128, NJ * E], f32, tag="lt")
        nc.vector.tensor_copy(lt[:], zt[:])
        lt3 = lt[:].rearrange("p (j e) -> p j e", e=E)

        def g(tag, shape=(128, NJ)):
            return cpool.tile(list(shape), f32, tag=tag, name=tag)

        m1 = g("g_m1")
        nc.vector.reduce_max(m1[:], lt3, axis=mybir.AxisListType.X)
        m1b = m1[:][:, :, None].broadcast_to([128, NJ, E])
        mask1 = g("g_mask1", (128, NJ * E))
        nc.vector.tensor_tensor(mask1[:].rearrange("p (j e) -> p j e", e=E),
                                lt3, m1b, op=Alu.is_equal)
        l2 = g("g_l2", (128, NJ * E))
        nc.vector.scalar_tensor_tensor(l2[:], mask1[:], -1e30, lt[:],
                                       op0=Alu.mult, op1=Alu.add)
        l23 = l2[:].rearrange("p (j e) -> p j e", e=E)
        m2 = g("g_m2")
        nc.vector.reduce_max(m2[:], l23, axis=mybir.AxisListType.X)
        mask2 = g("g_mask2", (128, NJ * E))
        nc.vector.tensor_tensor(mask2[:].rearrange("p (j e) -> p j e", e=E),
                                l23, m2[:][:, :, None].broadcast_to([128, NJ, E]),
                                op=Alu.is_equal)
        dd = g("g_d")
        nc.vector.tensor_tensor(dd[:], m2[:], m1[:], op=Alu.subtract)
        ed = g("g_ed")
        nc.scalar.activation(ed[:], dd[:], Act.Exp)
        den = g("g_den")
        nc.vector.tensor_scalar_add(den[:], ed[:], 1.0)
        w1 = g("g_w1")  # = 1/(1+exp(m2-m1))
        nc.vector.reciprocal(w1[:], den[:])
        w2 = g("g_w2")
        nc.vector.tensor_mul(w2[:], ed[:], w1[:])
        tmp1 = g("g_t1", (128, NJ * E))
        tmp2 = g("g_t2", (128, NJ * E))
        wts = g("g_wts", (128, NJ * E))
        nc.vector.tensor_tensor(tmp1[:].rearrange("p (j e) -> p j e", e=E),
                                mask1[:].rearrange("p (j e) -> p j e", e=E),
                                w1[:][:, :, None].broadcast_to([128, NJ, E]),
                                op=Alu.mult)
        nc.vector.tensor_tensor(tmp2[:].rearrange("p (j e) -> p j e", e=E),
                                mask2[:].rearrange("p (j e) -> p j e", e=E),
                                w2[:][:, :, None].broadcast_to([128, NJ, E]),
                                op=Alu.mult)
        nc.vector.tensor_add(wts[:], tmp1[:], tmp2[:])

        # router_probs = softmax(logits) and DMA out
        pz = g("g_pz", (128, NJ * E))
        nc.vector.tensor_tensor(pz[:].rearrange("p (j e) -> p j e", e=E),
                                lt3, m1b, op=Alu.subtract)
        pe_ = g("g_pe", (128, NJ * E))
        nc.scalar.activation(pe_[:], pz[:], Act.Exp)
        ps = g("g_ps")
        nc.vector.reduce_sum(ps[:], pe_[:].rearrange("p (j e) -> p j e", e=E),
                             axis=mybir.AxisListType.X)
        pr = g("g_pr")
        nc.vector.reciprocal(pr[:], ps[:])
        probs = g("g_probs", (128, NJ * E))
        nc.vector.tensor_tensor(probs[:].rearrange("p (j e) -> p j e", e=E),
                                pe_[:].rearrange("p (j e) -> p j e", e=E),
                                pr[:][:, :, None].broadcast_to([128, NJ, E]),
                                op=Alu.mult)
        nc.sync.dma_start(o_probs.rearrange("(j p) e -> p j e", p=128),
                          probs[:].rearrange("p (j e) -> p j e", e=E))

        # ---- expand gate weights to token granularity ----
        w_tok = dpool.tile([NT, E], f32, tag="w_tok")  # DRAM staging
        w_tok_r = w_tok[:].rearrange("(j p m) e -> p j m e", p=128, m=M)
        for m in range(M):
            nc.sync.dma_start(w_tok_r[:, :, m, :],
                              wts[:].rearrange("p (j e) -> p j e", e=E))
        w_tok_fm = cpool.tile([E, NT], f32, tag="w_tok_fm")
        nc.sync.dma_start(w_tok_fm[:], w_tok[:].rearrange("t e -> e t"))

        # ---- expert heads, feature-major, chunked over tokens ----
        for c in range(NCH):
            csl = ts(c, CH)
            acc = accpool.tile([FUT2 + 1, CH], f32, tag="acc")
            sp = s3pool.tile([1, CH], f32, tag="s3")
            hs1_t = {}
            for e in range(E):
                xs = x_fm[:, csl]
                # t1: [D,tok] -> [H,tok]
                h1 = h1pool.tile([128, 2, CH], f32r, tag="h1", name="h1")
                for m in range(2):
                    zt = zpool.tile([128, CH], f32, tag="z")
                    for s in range(CH // BCH):
                        nc.tensor.matmul(zt[:, ts(s, BCH)],
                                         wt1[e][:, ts(m, 128)],
                                         xs[:, ts(s, BCH)],
                                         start=True, stop=True)
                    nc.scalar.activation(h1[:, m, :], zt[:], Act.Gelu,
                                         bias=bt1[e][:, m : m + 1])
                # t2
                h2 = h2pool.tile([128, 2, CH], f32r, tag="h2")
                for m in range(2):
                    zt = zpool.tile([128, CH], f32, tag="z")
                    for s in range(CH // BCH):
                        for k in range(2):
                            nc.tensor.matmul(
                                zt[:, ts(s, BCH)],
                                wt2[e][:, 256 * k + 128 * m:
                                       256 * k + 128 * m + 128],
                                h1[:, k, ts(s, BCH)],
                                start=(k == 0), stop=(k == 1))
                    nc.scalar.activation(h2[:, m, :], zt[:], Act.Gelu,
                                         bias=bt2[e][:, m : m + 1])
                # gate-weight broadcast tile for this expert/chunk
                # (partition_broadcast needs its source at partition 0)
                wrow = tmppool.tile([1, CH], f32, tag="wrow", name="wrow",
                                    bufs=1)
                nc.sync.dma_start(wrow[:], w_tok_fm[e : e + 1, csl])
                wb = wbpool.tile([128, CH], f32, tag="wb")
                nc.gpsimd.partition_broadcast(wb[:], wrow[:], channels=128)
                # gate-weighted biases (fp32 matmul so w_tok_fm needs no
                # f32r rounding): rows 0:120 traj bias, row 120 score bias
                if e == 0:
                    bp = t3pool.tile([FUT2 + 1, CH], f32, tag="t3", name="bp")
                    for s in range(CH // BCH):
                        nc.tensor.matmul(bp[:, ts(s, BCH)], btst[:],
                                         w_tok_fm[:, csl][:, ts(s, BCH)],
                                         start=True, stop=True)
                    nc.vector.tensor_copy(acc[:], bp[:])
                # t3
                tp = t3pool.tile([FUT2, CH], f32, tag="t3")
                for s in range(CH // BCH):
                    for k in range(2):
                        nc.tensor.matmul(tp[:, ts(s, BCH)],
                                         wt3[e][:, ts(k, 120)],
                                         h2[:, k, ts(s, BCH)],
                                         start=(k == 0), stop=(k == 1))
                # combine: acc += tp * wb
                tm = tmppool.tile([FUT2, CH], f32, tag="tmp")
                nc.vector.tensor_mul(tm[:], tp[:], wb[0:FUT2, :])
                nc.gpsimd.tensor_add(acc[0:FUT2, :], acc[0:FUT2, :], tm[:])
                # s1
                hs1 = hs1pool.tile([128, CH], f32r, tag="hs1")
                zt = zpool.tile([128, CH], f32, tag="z")
                for s in range(CH // BCH):
                    nc.tensor.matmul(zt[:, ts(s, BCH)], ws1[e][:],
                                     xs[:, ts(s, BCH)], start=True, stop=True)
                nc.scalar.activation(hs1[:], zt[:], Act.Gelu, bias=bs1[e][:, 0:1])
                hs1_t[e] = hs1
                # s2
                zs2 = zpool.tile([64, CH], f32, tag="z", name="zs2")
                for s in range(CH // BCH):
                    nc.tensor.matmul(zs2[:, ts(s, BCH)], ws2[e][:],
                                     hs1[:, ts(s, BCH)], start=True, stop=True)
                hs2g = hs2pool.tile([64, CH], f32, tag="hs2g", name="hs2g",
                                    bufs=1)
                nc.scalar.activation(hs2g[:], zs2[:], Act.Gelu,
                                     bias=bs2[e][:, 0:1])
                # scale by gate weights (rounds to f32r)
                hs2s = hs2pool.tile([64, CH], f32r, tag="hs2s")
                nc.vector.tensor_mul(hs2s[:], hs2g[:], wb[0:64, :])
                # s3: accumulate scores over experts
                for s in range(CH // BCH):
                    nc.tensor.matmul(sp[:, ts(s, BCH)], ws3[e][:],
                                     hs2s[:, ts(s, BCH)],
                                     start=(e == 0), stop=(e == E - 1))
            # finalize chunk: scores out, traj transpose + out (PSUM can't DMA,
            # stage through SBUF)
            sc_bias = tmppool.tile([1, CH], f32, tag="sc_bias", name="sc_bias",
                                   bufs=1)
            nc.sync.dma_start(sc_bias[:], acc[FUT2 : FUT2 + 1, :])
            sc_sb = tmppool.tile([1, CH], f32, tag="sc_sb", name="sc_sb",
                                 bufs=1)
            nc.vector.tensor_add(sc_sb[:], sp[:], sc_bias[:])
            nc.sync.dma_start(o_scores_flat[ts(c, CH)][None, :], sc_sb[:])
            tt = zpool.tile([128, CH], f32, tag="z")
            for k in range(8):
                nc.tensor.transpose(tt[:, 128 * k : 128 * k + FUT2],
                                    acc[0:FUT2, ts(k, 128)], ident[0:FUT2, 0:FUT2])
            tt_sb = tmppool.tile([128, CH], f32, tag="tt_sb", name="tt_sb")
            nc.vector.tensor_copy(
                tt_sb[:].rearrange("p (k x) -> p k x", x=128)[:, :, 0:FUT2],
                tt[:].rearrange("p (k x) -> p k x", x=128)[:, :, 0:FUT2])
            nc.sync.dma_start(
                o_traj_flat.rearrange("(c k p) o -> c p k o", k=8, p=128)[c],
                tt_sb[:].rearrange("p (k x) -> p k x", x=128)[:, :, 0:FUT2],
            )

    nc.compile()
    return nc


def _get_nc():
    if "nc" not in _CACHE:
        _CACHE["nc"] = _build_nc()
    return _CACHE["nc"]


def kernel(**inputs):
    from concourse import bass_utils

    nc = _get_nc()
    mf = np.ascontiguousarray(inputs["mode_features"], dtype=np.float32)
    wnames = ["Wr1", "br1", "Wr2", "br2", "Wr3", "br3",
              "Wt1", "bt1", "Wt2", "bt2", "Wt3", "bt3",
              "Ws1", "bs1", "Ws2", "bs2", "Ws3", "bs3"]
    base = {n: np.ascontiguousarray(inputs[n], dtype=np.float32) for n in wnames}
    in_maps = []
    for i in range(NCORES):
        m = dict(base)
        m["mode_features"] = mf[i * B : (i + 1) * B]
        in_maps.append(m)

    res = bass_utils.run_bass_kernel_spmd(nc, in_maps, core_ids=list(range(NCORES)))

    traj = np.concatenate([r["out_traj"] for r in res.results], axis=0)
    scores = np.concatenate([r["out_scores"] for r in res.results], axis=0)
    probs = np.concatenate([r["out_probs"] for r in res.results], axis=0)

    trajectories = traj.reshape(B_FULL, M, FUT2 // 2, 2)
    # aux loss from router_probs (host glue on a 6-vector)
    avg = probs.astype(np.float64).mean(axis=0)
    entropy = -(avg * np.log(avg + 1e-8)).sum()
    aux = np.float32(-entropy * 0.01 + 0.01 * ((avg - 1.0 / E) ** 2).mean())
    return trajectories, scores, np.float32(aux), probs
